# revision 1
# baseline (speedup 1.0000x reference)
"""Trainium2 Bass kernel for the HexPlane-style decoder (nn_DecoderBase).

Math (B=1): six 3x3 SAME convs (64->16ch) + bias + ReLU + 2x nearest
upsample, channels-last, then broadcast Hadamard into
voxel[t, x, y, z, c] of shape [16, 64, 64, 32, 16] (f32, 128 MiB).

Sharding: X (64) split across 8 cores (8 x-values each; conv halos are
sliced host-side).  Per core the product factorizes as

  out[t,x,y,z,c] = M1[x,y,z,c] * ( ty[t,y,c] * Q[t,x,z,c] ),
  M1 = uxy*uxz*uyz,  Q = utx*utz.

Device layout: partition p = z0*64 + y (z = z0*16 + z1).  ty*Q is computed
by the TensorEngine as K=2 selector matmuls into PSUM (16 matmuls per t,
one per channel), so the VectorEngine runs a single fp32 tensor_tensor
pass (M1 * V) per t, overlapped with the 16 MiB/core output DMA.
"""

import numpy as np

T, XL, Y, Z, C = 16, 8, 64, 32, 16
NCORES = 8
CIN = 64

_CACHE = {}


def _build_program():
    from contextlib import ExitStack

    import concourse.bacc as bacc
    import concourse.bass as bass
    import concourse.mybir as mybir
    from concourse.tile import TileContext

    f32 = mybir.dt.float32
    bf16 = mybir.dt.bfloat16
    AF = mybir.ActivationFunctionType
    MUL = mybir.AluOpType.mult
    AP = bass.AP

    nc = bacc.Bacc()
    ctx = ExitStack()

    # ---- external IO ----
    ein = lambda name, shape: nc.dram_tensor(name, shape, f32, kind="ExternalInput")
    img_xy = ein("img_xy", [CIN, 206])
    img_xz = ein("img_xz", [CIN, 110])
    img_yz = ein("img_yz", [CIN, 614])
    img_tx = ein("img_tx", [CIN, 62])
    img_ty = ein("img_ty", [CIN, 342])
    img_tz = ein("img_tz", [CIN, 182])
    wr = ein("wr", [CIN, 6, 3, 3, 16])      # (cin, plane, dy, dx, cout)
    b_flat = ein("b_flat", [1, 96])
    b_t = ein("b_t", [16, 6])
    ones_in = ein("ones_in", [1, 128])
    out_d = nc.dram_tensor("out", [T, XL, Y, Z, C], f32, kind="ExternalOutput")

    # ---- DRAM scratch: upsampled channels-last conv outputs ----
    dtx = nc.dram_tensor("dtx", [T, XL, C], f32)     # (t, x, c)
    dtz = nc.dram_tensor("dtz", [T, Z, C], f32)      # (t, z, c)
    dty = nc.dram_tensor("dty", [16, 8, 32], f32)    # (c, t', y') pre-upsample
    dxy = nc.dram_tensor("dxy", [XL, Y, C], f32)     # (x, y, c)
    dxz = nc.dram_tensor("dxz", [XL, Z, C], f32)     # (x, z, c)
    dyz = nc.dram_tensor("dyz", [Y, Z, C], f32)      # (y, z, c)
    qd = nc.dram_tensor("qd", [T * XL * Z * C + 16], bf16)  # (t,x,z,c) flat +pad
    # raw conv-output dumps (flat [m*16], junk rows included)
    edump = {k: nc.dram_tensor(f"e_{k}", [m * 16], f32) for k, m in
             [("tx", 48), ("tz0", 72), ("tz1", 72), ("xy0", 68), ("xy1", 68),
              ("xz", 72), ("yz0", 126), ("yz1", 126), ("yz2", 126),
              ("yz3", 126), ("yz4", 72)]}

    with TileContext(nc) as tc:
        sb = lambda name, shape: ctx.enter_context(
            nc.sbuf_tensor(name, shape, f32))
        # inputs
        i_xy, i_xz, i_yz = (sb("i_xy", [CIN, 206]), sb("i_xz", [CIN, 110]),
                            sb("i_yz", [CIN, 614]))
        i_tx, i_ty, i_tz = (sb("i_tx", [CIN, 62]), sb("i_ty", [CIN, 342]),
                            sb("i_tz", [CIN, 182]))
        w_sb, bf_sb = sb("w_sb", [CIN, 864]), sb("bf_sb", [1, 96])
        bt_sb, on_sb = sb("bt_sb", [16, 6]), sb("on_sb", [1, 128])
        # voxel operands
        utx = sb("utx", [128, 16])        # p=(t,x): c
        utz = sb("utz", [128, 512])       # p=(t,x): (z,c)
        q_s = ctx.enter_context(nc.sbuf_tensor("q_s", [128, 512], bf16))
        uxy = sb("uxy", [128, 128])       # p=(z0,y): (x,c)
        uxz = sb("uxz", [128, 2048])      # p=(z0,y): (x,z1,c)
        uyz = sb("uyz", [128, 256])       # p=(z0,y): (z1,c)
        m1a = sb("m1a", [128, 2048])
        m1 = sb("m1", [128, 2048])
        ty_raw = sb("ty_raw", [1, 4096])   # (c', t', y') flat dty copy
        ty_all = ctx.enter_context(nc.sbuf_tensor("ty_all", [1, 8192], bf16))

        # ---------- phase A: input loads (Q-path planes first) ----------
        for dst, srca in ((i_tx, img_tx), (i_tz, img_tz), (i_ty, img_ty)):
            nc.sync.dma_start(dst[:], srca[:])
        nc.sync.dma_start(w_sb[:], wr[:].rearrange("a b c d e -> a (b c d e)"))
        nc.sync.dma_start(bf_sb[:], b_flat[:])
        nc.sync.dma_start(bt_sb[:], b_t[:])
        nc.sync.dma_start(on_sb[:], ones_in[:])
        for dst, srca in ((i_xy, img_xy), (i_xz, img_xz), (i_yz, img_yz)):
            nc.sync.dma_start(dst[:], srca[:])

        # ---------- phase B: convolutions + staging, Q-path first ----------
        def wslice(i, dy, dx):
            off = ((i * 3 + dy) * 3 + dx) * 16
            return w_sb[:, off:off + 16]

        conv_pool_cm = tc.tile_pool(name="convpsum", bufs=2, space="PSUM")
        conv_pool = conv_pool_cm.__enter__()

        conv_outs = {}

        def conv_spatial(i, img, fsz, wp, rows, row0, tag):
            # Full-width contiguous windows (stationary AP must be 1-D):
            # out flat m = r*wp + col; junk at cols wp-2, wp-1.
            m = rows * wp
            psum = conv_pool.tile([m, 16], f32, name=f"cp_{tag}", tag="cp")
            for dy in range(3):
                for dx in range(3):
                    lhsT = AP(img, (row0 + dy) * wp + dx, [[fsz, CIN], [1, m]])
                    nc.tensor.matmul(psum, lhsT, wslice(i, dy, dx),
                                     start=(dy == 0 and dx == 0), stop=False)
            nc.tensor.matmul(psum, on_sb[:, :m],
                             bf_sb[:, i * 16:i * 16 + 16], start=False, stop=True)
            out_sb = sb(f"c_{tag}", [m, 16])
            nc.scalar.activation(out_sb[:], psum, AF.Relu)
            conv_outs[tag] = out_sb

        dump_insts = {}
        stage_insts = {}

        def dump(eng, k):
            dump_insts[k] = eng.dma_start(edump[k][:], conv_outs[k][:])

        def stage(eng, key, srck, dst_ap, src_ap):
            inst = eng.dma_start(dst_ap, src_ap)
            if srck is not None:
                bass._add_dep_helper(inst.ins, dump_insts[srck].ins,
                                     reason=f"dump {srck}")
            stage_insts.setdefault(key, []).append(inst)

        def after_stage(key, inst):
            for si in stage_insts[key]:
                bass._add_dep_helper(inst.ins, si.ins, reason=f"raw {key}")
            return inst

        # --- tx ---
        conv_spatial(3, i_tx, 62, 6, 8, 0, "tx")              # m=48
        dump(nc.sync, "tx")
        for rt in range(2):
            for rx in range(2):
                stage(nc.sync, "dtx", "tx",
                      AP(dtx, rt * XL * C + rx * C,
                         [[2 * XL * C, 8], [2 * C, 4], [1, 16]]),
                      AP(edump["tx"], 0, [[96, 8], [16, 4], [1, 16]]))
        # --- tz ---
        conv_spatial(5, i_tz, 182, 18, 4, 0, "tz0")           # m=72
        conv_spatial(5, i_tz, 182, 18, 4, 4, "tz1")
        for k in range(2):
            dump(nc.sync, f"tz{k}")
            for rt in range(2):
                for rz in range(2):
                    stage(nc.sync, "dtz", f"tz{k}",
                          AP(dtz, (8 * k + rt) * Z * C + rz * C,
                             [[2 * Z * C, 4], [2 * C, 16], [1, 16]]),
                          AP(edump[f"tz{k}"], 0, [[288, 4], [16, 16], [1, 16]]))
        # --- ty (cout-partition conv; W stationary) ---
        psum_ty = conv_pool.tile([16, 272], f32, name="cp_ty", tag="cpty")
        for dy in range(3):
            for dx in range(3):
                rhs = AP(i_ty, dy * 34 + dx, [[342, CIN], [1, 272]])
                nc.tensor.matmul(psum_ty, wslice(4, dy, dx), rhs,
                                 start=(dy == 0 and dx == 0),
                                 stop=(dy == 2 and dx == 2))
        cty = sb("cty", [16, 272])
        nc.scalar.activation(cty[:], psum_ty, AF.Relu, bias=bt_sb[:, 4:5])
        stage_insts["dty"] = [nc.sync.dma_start(
            dty[:].rearrange("a b c -> a (b c)"),
            AP(cty, 0, [[272, 16], [34, 8], [1, 32]]))]

        # --- Q = utx * utz ---
        after_stage("dtx", nc.sync.dma_start(
            utx[:], AP(dtx, 0, [[16, 128], [1, 16]])))
        after_stage("dtz", nc.sync.dma_start(
            utz[:], AP(dtz, 0, [[Z * C, 16], [0, 8], [1, Z * C]])))
        nc.vector.tensor_tensor(
            q_s[:], utz[:], AP(utx, 0, [[16, 128], [0, 32], [1, 16]]), MUL)
        q_store = nc.sync.dma_start(AP(qd, 0, [[512, 128], [1, 512]]), q_s[:])

        # --- ty_all strip (vector copies; y upsampled, cast to bf16) ---
        after_stage("dty", nc.sync.dma_start(ty_raw[:], dty[:]))
        for tp in range(8):
            nc.vector.tensor_copy(
                AP(ty_all, tp * 1024, [[8192, 1], [1, 1024]]),
                AP(ty_raw, tp * 32, [[4096, 1], [256, 16], [1, 32], [0, 2]]))

        # --- M1-path planes (staging DMAs on the scalar HWDGE engine) ---
        conv_spatial(0, i_xy, 206, 34, 2, 0, "xy0")           # m=68
        conv_spatial(0, i_xy, 206, 34, 2, 2, "xy1")
        for k in range(2):
            dump(nc.scalar, f"xy{k}")
            for rx in range(2):
                for ry in range(2):
                    stage(nc.scalar, "dxy", f"xy{k}",
                          AP(dxy, (4 * k + rx) * Y * C + ry * C,
                             [[2 * Y * C, 2], [2 * C, 32], [1, 16]]),
                          AP(edump[f"xy{k}"], 0, [[544, 2], [16, 32], [1, 16]]))
        conv_spatial(1, i_xz, 110, 18, 4, 0, "xz")            # m=72
        dump(nc.scalar, "xz")
        for rx in range(2):
            for rz in range(2):
                stage(nc.scalar, "dxz", "xz",
                      AP(dxz, rx * Z * C + rz * C,
                         [[2 * Z * C, 4], [2 * C, 16], [1, 16]]),
                      AP(edump["xz"], 0, [[288, 4], [16, 16], [1, 16]]))
        yz_rows = [(0, 7), (7, 7), (14, 7), (21, 7), (28, 4)]
        for bb, (r0, nr) in enumerate(yz_rows):
            conv_spatial(2, i_yz, 614, 18, nr, r0, f"yz{bb}")
            dump(nc.scalar, f"yz{bb}")
            for ry in range(2):
                for rz in range(2):
                    stage(nc.scalar, "dyz", f"yz{bb}",
                          AP(dyz, (2 * r0 + ry) * Z * C + rz * C,
                             [[2 * Z * C, nr], [2 * C, 16], [1, 16]]),
                          AP(edump[f"yz{bb}"], 0, [[288, nr], [16, 16], [1, 16]]))

        conv_pool_cm.__exit__(None, None, None)

        # ---------- M1 operand loads + build ----------
        for z0 in range(2):
            after_stage("dxy", nc.scalar.dma_start(
                uxy[z0 * 64:(z0 + 1) * 64, :],
                AP(dxy, 0, [[C, 64], [Y * C, 8], [1, 16]])))
            after_stage("dxz", nc.scalar.dma_start(
                uxz[z0 * 64:(z0 + 1) * 64, :],
                AP(dxz, z0 * 16 * C, [[0, 64], [Z * C, 8], [1, 256]])))
        after_stage("dyz", nc.scalar.dma_start(
            uyz[:], AP(dyz, 0, [[16 * C, 2], [Z * C, 64], [1, 256]])))

        nc.vector.tensor_tensor(
            m1a[:], uxz[:], AP(uyz, 0, [[256, 128], [0, 8], [1, 256]]), MUL)
        nc.vector.tensor_tensor(
            m1[:], m1a[:], AP(uxy, 0, [[128, 128], [16, 8], [0, 16], [1, 16]]),
            MUL)

        # ---------- phase E: per-t voxel ----------
        from contextlib import ExitStack as _ES
        pool_ctx = _ES()
        qz_pool = pool_ctx.enter_context(tc.tile_pool(name="qz", bufs=1))
        v_pool = pool_ctx.enter_context(
            tc.tile_pool(name="vps", bufs=2, space="PSUM"))
        out_pool = pool_ctx.enter_context(tc.tile_pool(name="outsb", bufs=3))

        for tg in range(4):
            # Q rows for 4 t's x 2 z0-halves; "o" copies shifted one element
            # so odd-channel slices read 4-byte-aligned bases
            qzh, qzo = [], []
            for z0 in range(2):
                qz = qz_pool.tile([1, 8192], bf16, name=f"qz{z0}", tag=f"qz{z0}")
                bass._add_dep_helper(
                    nc.sync.dma_start(
                        qz, AP(qd, tg * 4 * XL * Z * C + z0 * 16 * C,
                               [[XL * Z * C, 4], [Z * C, 8], [1, 256]])).ins,
                    q_store.ins, reason="raw qd")
                qzh.append(qz)
                qo = qz_pool.tile([1, 8192], bf16, name=f"qo{z0}", tag=f"qo{z0}")
                bass._add_dep_helper(
                    nc.sync.dma_start(
                        qo, AP(qd, tg * 4 * XL * Z * C + z0 * 16 * C + 1,
                               [[XL * Z * C, 4], [Z * C, 8], [1, 256]])).ins,
                    q_store.ins, reason="raw qd")
                qzo.append(qo)

            for ti in range(4):
                t = tg * 4 + ti
                v = v_pool.tile([128, 2048], f32, name="v", tag="v")
                vp = v.ap[0][0]
                for z0 in range(2):
                    for cp in range(16):
                        lhsT = AP(ty_all, (t // 2) * 1024 + cp * 64,
                                  [[8192, 1], [1, 64]])
                        if cp % 2 == 0:
                            rhs = AP(qzh[z0].tensor,
                                     qzh[z0].offset + ti * 2048 + cp,
                                     [[8192, 1], [256, 8], [16, 16]])
                        else:
                            rhs = AP(qzo[z0].tensor,
                                     qzo[z0].offset + ti * 2048 + cp - 1,
                                     [[8192, 1], [256, 8], [16, 16]])
                        nc.tensor.matmul(
                            v[z0 * 64:(z0 + 1) * 64, cp * 128:(cp + 1) * 128],
                            lhsT, rhs, start=True, stop=True)

                o = out_pool.tile([128, 2048], f32, name="o", tag="o")
                op = o.ap[0][0]
                nc.vector.tensor_tensor(
                    AP(o.tensor, o.offset,
                       [[op, 128], [256, 8], [16, 16], [1, 16]]),
                    AP(m1, 0, [[2048, 128], [256, 8], [16, 16], [1, 16]]),
                    AP(v.tensor, v.offset,
                       [[vp, 128], [16, 8], [1, 16], [128, 16]]),
                    MUL)
                for z0 in range(2):
                    dst = AP(out_d, t * XL * Y * Z * C + z0 * 16 * C,
                             [[Z * C, 64], [Y * Z * C, 8], [1, 256]])
                    nc.scalar.dma_start(dst, o[z0 * 64:(z0 + 1) * 64, :])

        pool_ctx.close()

    nc.compile()
    return nc, ctx


def _prep_inputs(plane_xy, plane_xz, plane_yz, plane_tx, plane_ty, plane_tz, W, b):
    """Host-side slicing/padding/transposition. Returns per-core input maps."""
    f32 = np.float32
    xy = np.asarray(plane_xy, f32)[0]  # [64, X'32, Y'32]
    xz = np.asarray(plane_xz, f32)[0]  # [64, X'32, Z'16]
    yz = np.asarray(plane_yz, f32)[0]  # [64, Y'32, Z'16]
    tx = np.asarray(plane_tx, f32)[0]  # [64, T'8,  X'32]
    ty = np.asarray(plane_ty, f32)[0]  # [64, T'8,  Y'32]
    tz = np.asarray(plane_tz, f32)[0]  # [64, T'8,  Z'16]
    W = np.asarray(W, f32)             # [6, 16, 64, 3, 3]
    b = np.asarray(b, f32)             # [6, 16]

    wr = np.ascontiguousarray(W.transpose(2, 0, 3, 4, 1))  # (ci, i, dy, dx, co)
    b_flat = np.ascontiguousarray(b.reshape(1, 96))
    b_t = np.ascontiguousarray(b.T)
    ones = np.ones((1, 128), f32)

    def flat2(p):
        q = p.reshape(p.shape[0], -1)
        return np.ascontiguousarray(
            np.pad(q, ((0, 0), (0, 2))))

    img_yz = flat2(np.pad(yz, ((0, 0), (1, 1), (1, 1))))
    img_ty = flat2(np.pad(ty, ((0, 0), (1, 1), (1, 1))))
    img_tz = flat2(np.pad(tz, ((0, 0), (1, 1), (1, 1))))

    def row_halo(p, x0h):
        out = np.zeros((p.shape[0], 6, p.shape[2]), f32)
        lo = x0h - 1
        s0, s1 = max(lo, 0), min(lo + 6, p.shape[1])
        out[:, s0 - lo:s0 - lo + (s1 - s0), :] = p[:, s0:s1, :]
        return out

    def col_halo(p, x0h):
        out = np.zeros((p.shape[0], p.shape[1], 6), f32)
        lo = x0h - 1
        s0, s1 = max(lo, 0), min(lo + 6, p.shape[2])
        out[:, :, s0 - lo:s0 - lo + (s1 - s0)] = p[:, :, s0:s1]
        return out

    in_maps = []
    for k in range(NCORES):
        x0h = 4 * k
        in_maps.append({
            "img_xy": flat2(np.pad(row_halo(xy, x0h), ((0, 0), (0, 0), (1, 1)))),
            "img_xz": flat2(np.pad(row_halo(xz, x0h), ((0, 0), (0, 0), (1, 1)))),
            "img_yz": img_yz,
            "img_tx": flat2(np.pad(col_halo(tx, x0h), ((0, 0), (1, 1), (0, 0)))),
            "img_ty": img_ty,
            "img_tz": img_tz,
            "wr": wr,
            "b_flat": b_flat,
            "b_t": b_t,
            "ones_in": ones,
        })
    return in_maps


def kernel(plane_xy, plane_xz, plane_yz, plane_tx, plane_ty, plane_tz, W, b):
    from concourse.bass_utils import run_bass_kernel_spmd

    if "nc" not in _CACHE:
        _CACHE["nc"], _CACHE["ctx"] = _build_program()
    nc = _CACHE["nc"]

    in_maps = _prep_inputs(plane_xy, plane_xz, plane_yz, plane_tx, plane_ty,
                           plane_tz, W, b)
    res = run_bass_kernel_spmd(nc, in_maps, list(range(NCORES)))
    slices = [res.results[k]["out"] for k in range(NCORES)]
    full = np.concatenate(slices, axis=1)  # [T, 64, Y, Z, C]
    return full[None].astype(np.float32)



# revision 10
# speedup vs baseline: 1.7361x; 1.7361x over previous
"""Trainium2 Bass kernel for the HexPlane-style decoder (nn_DecoderBase).

Math (B=1): six 3x3 SAME convs (64->16ch) + bias + ReLU + 2x nearest
upsample, channels-last, then broadcast Hadamard into
voxel[t, x, y, z, c] of shape [16, 64, 64, 32, 16] (f32, 128 MiB).

Key observation: every axis of the voxel (t, x, y, z) is 2x
nearest-upsampled, so out[t,x,y,z,c] depends only on
(t//2, x//2, y//2, z//2, c) -- only 1/16 of the output is unique.
We compute just the unique block per core and let the output DMAs
duplicate it on the way to HBM.

Sharding: X (64) split across 8 cores -> 4 unique x2-values per core
(conv halos sliced host-side).  Per core, with partitions p=(x2,y2):

  out[t2,x2,y2,z2,c] = M1[p,(z2,c)] * ty[t2,y2,c] * Q[t2,x2,z2,c]
  M1 = uxy*uxz*uyz (pre-upsample conv outs),  Q = utx*utz.

All cross-partition broadcasts are done by tiny replicated DMA loads
from conv-output dumps in DRAM (0-stride partition dims), so the whole
voxel phase is a handful of VectorE tensor_tensor ops; no PE matmuls
outside the convolutions.  Each unique [128, 1024] f32 tile is stored
4x (t-dup x x-dup) with y/z duplication folded into the DMA access
patterns (4 KiB contiguous runs).
"""

import numpy as np

T2, X2, Y2, Z2, C = 8, 4, 32, 16, 16
NCORES = 8
CIN = 64

_CACHE = {}


def _build_program():
    from contextlib import ExitStack

    import concourse.bacc as bacc
    import concourse.bass as bass
    import concourse.mybir as mybir
    from concourse.tile import TileContext

    f32 = mybir.dt.float32
    AF = mybir.ActivationFunctionType
    MUL = mybir.AluOpType.mult
    AP = bass.AP

    nc = bacc.Bacc()
    ctx = ExitStack()

    # ---- external IO ----
    ein = lambda name, shape: nc.dram_tensor(name, shape, f32, kind="ExternalInput")
    img_xyT = ein("img_xyT", [CIN, 206])  # transposed: y-pad(34) x x-halo(6) + 2
    img_xz = ein("img_xz", [CIN, 110])    # x-halo rows(6) x z-pad(18) + 2
    img_yz = ein("img_yz", [CIN, 614])    # y-pad(34) x z-pad(18) + 2
    img_tx = ein("img_tx", [CIN, 62])     # t-pad(10) x x-halo cols(6) + 2
    img_tyT = ein("img_tyT", [CIN, 342])  # transposed: y-pad(34) x t-pad(10) + 2
    img_tz = ein("img_tz", [CIN, 182])    # t-pad(10) x z-pad(18) + 2
    wr = ein("wr", [CIN, 864])            # (cin, plane, dy, dx, cout) flat
    b_flat = ein("b_flat", [1, 96])
    ones_in = ein("ones_in", [1, 128])
    out_d = nc.dram_tensor("out", [2 * T2, 2 * X2, 2 * Y2, 2 * Z2, C], f32,
                           kind="ExternalOutput")

    # ---- DRAM scratch: raw conv-output dumps (flat [m*16]) ----
    yz_rows = [(0, 7), (7, 7), (14, 7), (21, 7), (28, 4)]
    ty_rows = [(0, 12), (12, 12), (24, 8)]
    edump = {}
    xy_rows = [(0, 21), (21, 11)]
    for k, m in ([("xz", 72), ("tx", 48), ("tz0", 72), ("tz1", 72)]
                 + [(f"xy{b}", nr * 6) for b, (r0, nr) in enumerate(xy_rows)]
                 + [(f"yz{b}", nr * 18) for b, (r0, nr) in enumerate(yz_rows)]
                 + [(f"ty{b}", nr * 10) for b, (r0, nr) in enumerate(ty_rows)]):
        edump[k] = nc.dram_tensor(f"e_{k}", [m * 16], f32)
    quD = nc.dram_tensor("quD", [32 * 256], f32)  # (x2, t2, z2, c) flat

    with TileContext(nc) as tc:
        sb = lambda name, shape: ctx.enter_context(
            nc.sbuf_tensor(name, shape, f32))
        # inputs
        i_xyT, i_xz, i_yz = (sb("i_xyT", [CIN, 206]), sb("i_xz", [CIN, 110]),
                             sb("i_yz", [CIN, 614]))
        i_tx, i_tyT, i_tz = (sb("i_tx", [CIN, 62]), sb("i_tyT", [CIN, 342]),
                             sb("i_tz", [CIN, 182]))
        w_sb, bf_sb, on_sb = (sb("w_sb", [CIN, 864]), sb("bf_sb", [1, 96]),
                              sb("on_sb", [1, 128]))
        # voxel operands (partitions p = y2*4 + x2 unless noted)
        uxy_sb = sb("uxy_sb", [128, 16])      # p: c
        uxz_rep = sb("uxz_rep", [128, 256])   # p: (z2, c)  [rep over y2]
        uyz_rep = sb("uyz_rep", [128, 256])   # p: (z2, c)  [rep over x2]
        uty_rep = sb("uty_rep", [128, 128])   # p: (t2, c)  [rep over x2]
        qu_rep = sb("qu_rep", [128, 2048])    # p: (t2, z2, c)  [rep over y2]
        utx_sb = sb("utx_sb", [32, 16])       # p=(t2,x2): c
        utz_sb = sb("utz_sb", [32, 256])      # p=(t2,x2): (z2, c)
        qu_sb = sb("qu_sb", [32, 256])        # p=(t2,x2): (z2, c)
        m1a = sb("m1a", [128, 256])
        m1u = sb("m1u", [128, 256])
        tmp_all = sb("tmp_all", [128, 2048])  # p: (t2, z2, c) = m1u * ty

        # ---------- phase A: input loads ----------
        for dst, srca in ((i_tx, img_tx), (i_tz, img_tz), (i_tyT, img_tyT)):
            nc.sync.dma_start(dst[:], srca[:])
        nc.sync.dma_start(w_sb[:], wr[:])
        nc.sync.dma_start(bf_sb[:], b_flat[:])
        nc.sync.dma_start(on_sb[:], ones_in[:])
        for dst, srca in ((i_xyT, img_xyT), (i_xz, img_xz), (i_yz, img_yz)):
            nc.scalar.dma_start(dst[:], srca[:])

        # ---------- phase B: convolutions ----------
        def wslice(i, dy, dx):
            off = ((i * 3 + dy) * 3 + dx) * 16
            return w_sb[:, off:off + 16]

        conv_pool_cm = tc.tile_pool(name="convpsum", bufs=2, space="PSUM")
        conv_pool = conv_pool_cm.__enter__()

        conv_outs = {}

        def conv_spatial(i, img, fsz, wp, rows, row0, tag):
            # Full-width contiguous windows; junk at cols wp-2, wp-1.
            m = rows * wp
            psum = conv_pool.tile([m, 16], f32, name=f"cp_{tag}", tag="cp")
            for dy in range(3):
                for dx in range(3):
                    lhsT = AP(img, (row0 + dy) * wp + dx, [[fsz, CIN], [1, m]])
                    nc.tensor.matmul(psum, lhsT, wslice(i, dy, dx),
                                     start=(dy == 0 and dx == 0), stop=False)
            nc.tensor.matmul(psum, on_sb[:, :m],
                             bf_sb[:, i * 16:i * 16 + 16], start=False,
                             stop=True)
            out_sb = sb(f"c_{tag}", [m, 16])
            nc.scalar.activation(out_sb[:], psum, AF.Relu)
            conv_outs[tag] = out_sb

        dump_insts = {}

        def dump(eng, k):
            dump_insts[k] = eng.dma_start(edump[k][:], conv_outs[k][:])

        def reload(eng, deps, dst_ap, src_ap):
            inst = eng.dma_start(dst_ap, src_ap)
            for d in deps:
                bass._add_dep_helper(inst.ins, dump_insts[d].ins,
                                     reason=f"raw {d}")
            return inst

        # --- Q path first: tx, tz, then ty ---
        conv_spatial(3, i_tx, 62, 6, 8, 0, "tx")               # m=48
        dump(nc.sync, "tx")
        for k in range(2):
            conv_spatial(5, i_tz, 182, 18, 4, 4 * k, f"tz{k}")  # m=72
            dump(nc.sync, f"tz{k}")
        for b, (r0, nr) in enumerate(ty_rows):
            conv_spatial(4, i_tyT, 342, 10, nr, r0, f"ty{b}")
            dump(nc.sync, f"ty{b}")

        # Q = utx * utz on p=(t2, x2)
        reload(nc.sync, ["tx"], utx_sb[:],
               AP(edump["tx"], 0, [[96, 8], [16, 4], [1, 16]]))
        for k in range(2):
            reload(nc.sync, [f"tz{k}"],
                   AP(utz_sb, k * 16 * 256, [[256, 16], [1, 256]]),
                   AP(edump[f"tz{k}"], 0, [[288, 4], [0, 4], [1, 256]]))
        nc.vector.tensor_tensor(
            qu_sb[:], utz_sb[:], AP(utx_sb, 0, [[16, 32], [0, 16], [1, 16]]),
            MUL)
        # store as (x2, t2, z2, c) so the replicated reload is contiguous
        qu_store = nc.sync.dma_start(
            AP(quD, 0, [[256, 8], [2048, 4], [1, 256]]), qu_sb[:])
        qu_load = nc.sync.dma_start(
            qu_rep[:], AP(quD, 0, [[0, 32], [2048, 4], [1, 2048]]))
        bass._add_dep_helper(qu_load.ins, qu_store.ins, reason="raw quD")

        # uty_rep[p=(y2,x2), (t2, c)] from transposed-ty dumps
        for b, (r0, nr) in enumerate(ty_rows):
            reload(nc.sync, [f"ty{b}"],
                   AP(uty_rep, r0 * 4 * 128, [[128, 4 * nr], [1, 128]]),
                   AP(edump[f"ty{b}"], 0, [[160, nr], [0, 4], [1, 128]]))

        # --- M1 path: xy (transposed plane -> y2-major rows), xz, yz ---
        for b, (r0, nr) in enumerate(xy_rows):
            conv_spatial(0, i_xyT, 206, 6, nr, r0, f"xy{b}")
            dump(nc.scalar, f"xy{b}")
            reload(nc.scalar, [f"xy{b}"],
                   AP(uxy_sb, r0 * 4 * 16, [[16, 4 * nr], [1, 16]]),
                   AP(edump[f"xy{b}"], 0, [[96, nr], [16, 4], [1, 16]]))

        conv_spatial(1, i_xz, 110, 18, 4, 0, "xz")             # m=72
        dump(nc.scalar, "xz")
        reload(nc.scalar, ["xz"], uxz_rep[:],
               AP(edump["xz"], 0, [[0, 32], [288, 4], [1, 256]]))
        for b, (r0, nr) in enumerate(yz_rows):
            conv_spatial(2, i_yz, 614, 18, nr, r0, f"yz{b}")
            dump(nc.scalar, f"yz{b}")
            reload(nc.scalar, [f"yz{b}"],
                   AP(uyz_rep, r0 * 4 * 256, [[256, 4 * nr], [1, 256]]),
                   AP(edump[f"yz{b}"], 0, [[288, nr], [0, 4], [1, 256]]))

        conv_pool_cm.__exit__(None, None, None)

        # ---------- phase C: M1 and ty products ----------
        nc.vector.tensor_tensor(m1a[:], uxz_rep[:], uyz_rep[:], MUL)
        nc.vector.tensor_tensor(
            m1u[:], m1a[:], AP(uxy_sb, 0, [[16, 128], [0, 16], [1, 16]]), MUL)
        # tmp_all[p, (t2, z2, c)] = m1u[p, (z2, c)] * uty_rep[p, (t2, c)]
        nc.vector.tensor_tensor(
            tmp_all[:],
            AP(m1u, 0, [[256, 128], [0, 8], [16, 16], [1, 16]]),
            AP(uty_rep, 0, [[128, 128], [16, 8], [0, 16], [1, 16]]), MUL)

        # ---------- phase D: per-t2 voxel tiles + duplicated stores ----------
        from contextlib import ExitStack as _ES
        pool_ctx = _ES()
        out_pool = pool_ctx.enter_context(tc.tile_pool(name="outsb", bufs=3))

        for t2 in range(T2):
            o = out_pool.tile([128, 1024], f32, name="o", tag="o")
            op = o.ap[0][0]
            # o[p, (z2, zd, c)] = tmp_all[p, t2, z2, c] * qu_rep[p, t2, z2, c]
            nc.vector.tensor_tensor(
                AP(o.tensor, o.offset, [[op, 128], [32, 16], [16, 2], [1, 16]]),
                AP(tmp_all, t2 * 256, [[2048, 128], [16, 16], [0, 2], [1, 16]]),
                AP(qu_rep, t2 * 256, [[2048, 128], [16, 16], [0, 2], [1, 16]]),
                MUL)
            # duplicate the (z, c) half-row for the y-duplication run
            nc.vector.tensor_copy(
                AP(o.tensor, o.offset + 512, [[op, 128], [1, 512]]),
                AP(o.tensor, o.offset, [[op, 128], [1, 512]]))
            for td in range(2):
                for xd in range(2):
                    eng = nc.sync if (td * 2 + xd) % 2 == 0 else nc.scalar
                    dst = AP(out_d,
                             (2 * t2 + td) * 262144 + xd * 32768,
                             [[1024, 32], [65536, 4], [1, 1024]])
                    eng.dma_start(dst, o[:])

        pool_ctx.close()

    nc.compile()
    return nc, ctx


def _prep_inputs(plane_xy, plane_xz, plane_yz, plane_tx, plane_ty, plane_tz,
                 W, b):
    """Host-side slicing/padding/transposition. Returns per-core input maps."""
    f32 = np.float32
    xy = np.asarray(plane_xy, f32)[0]  # [64, X'32, Y'32]
    xz = np.asarray(plane_xz, f32)[0]  # [64, X'32, Z'16]
    yz = np.asarray(plane_yz, f32)[0]  # [64, Y'32, Z'16]
    tx = np.asarray(plane_tx, f32)[0]  # [64, T'8,  X'32]
    ty = np.asarray(plane_ty, f32)[0]  # [64, T'8,  Y'32]
    tz = np.asarray(plane_tz, f32)[0]  # [64, T'8,  Z'16]
    W = np.asarray(W, f32)             # [6, 16, 64, 3, 3]
    b = np.asarray(b, f32)             # [6, 16]

    # xy and ty are convolved on transposed planes -> swap their 3x3 taps
    W2 = W.copy()
    W2[0] = W[0].transpose(0, 1, 3, 2)
    W2[4] = W[4].transpose(0, 1, 3, 2)
    wr = np.ascontiguousarray(
        W2.transpose(2, 0, 3, 4, 1).reshape(CIN, 864))   # (ci,i,dy,dx,co)
    b_flat = np.ascontiguousarray(b.reshape(1, 96))
    ones = np.ones((1, 128), f32)

    def flat2(p):
        q = p.reshape(p.shape[0], -1)
        return np.ascontiguousarray(np.pad(q, ((0, 0), (0, 2))))

    img_yz = flat2(np.pad(yz, ((0, 0), (1, 1), (1, 1))))
    img_tyT = flat2(np.pad(ty.transpose(0, 2, 1), ((0, 0), (1, 1), (1, 1))))
    img_tz = flat2(np.pad(tz, ((0, 0), (1, 1), (1, 1))))

    def row_halo(p, x0h):
        out = np.zeros((p.shape[0], 6, p.shape[2]), f32)
        lo = x0h - 1
        s0, s1 = max(lo, 0), min(lo + 6, p.shape[1])
        out[:, s0 - lo:s0 - lo + (s1 - s0), :] = p[:, s0:s1, :]
        return out

    def col_halo(p, x0h):
        out = np.zeros((p.shape[0], p.shape[1], 6), f32)
        lo = x0h - 1
        s0, s1 = max(lo, 0), min(lo + 6, p.shape[2])
        out[:, :, s0 - lo:s0 - lo + (s1 - s0)] = p[:, :, s0:s1]
        return out

    in_maps = []
    for k in range(NCORES):
        x0h = 4 * k
        in_maps.append({
            "img_xyT": flat2(np.pad(col_halo(xy.transpose(0, 2, 1), x0h),
                                     ((0, 0), (1, 1), (0, 0)))),
            "img_xz": flat2(np.pad(row_halo(xz, x0h), ((0, 0), (0, 0), (1, 1)))),
            "img_yz": img_yz,
            "img_tx": flat2(np.pad(col_halo(tx, x0h), ((0, 0), (1, 1), (0, 0)))),
            "img_tyT": img_tyT,
            "img_tz": img_tz,
            "wr": wr,
            "b_flat": b_flat,
            "ones_in": ones,
        })
    return in_maps


def kernel(plane_xy, plane_xz, plane_yz, plane_tx, plane_ty, plane_tz, W, b):
    from concourse.bass_utils import run_bass_kernel_spmd

    if "nc" not in _CACHE:
        _CACHE["nc"], _CACHE["ctx"] = _build_program()
    nc = _CACHE["nc"]

    in_maps = _prep_inputs(plane_xy, plane_xz, plane_yz, plane_tx, plane_ty,
                           plane_tz, W, b)
    res = run_bass_kernel_spmd(nc, in_maps, list(range(NCORES)))
    slices = [res.results[k]["out"] for k in range(NCORES)]
    full = np.concatenate(slices, axis=1)  # [T, 64, Y, Z, C]
    return full[None].astype(np.float32)


# revision 12
# speedup vs baseline: 2.4237x; 1.3960x over previous
"""Trainium2 Bass kernel for the HexPlane-style decoder (nn_DecoderBase).

Math (B=1): six 3x3 SAME convs (64->16ch) + bias + ReLU + 2x nearest
upsample, channels-last, then broadcast Hadamard into
voxel[t, x, y, z, c] of shape [16, 64, 64, 32, 16] (f32, 128 MiB).

Key observation: every axis of the voxel (t, x, y, z) is 2x
nearest-upsampled, so out[t,x,y,z,c] depends only on
(t//2, x//2, y//2, z//2, c) -- only 1/16 of the output is unique.
We compute just the unique block per core and let the output DMAs
duplicate it on the way to HBM.

Sharding: X (64) split across 8 cores -> 4 unique x2-values per core
(conv halos sliced host-side).  Per core, with partitions p=(x2,y2):

  out[t2,x2,y2,z2,c] = M1[p,(z2,c)] * ty[t2,y2,c] * Q[t2,x2,z2,c]
  M1 = uxy*uxz*uyz (pre-upsample conv outs),  Q = utx*utz.

All cross-partition broadcasts are done by tiny replicated DMA loads
from conv-output dumps in DRAM (0-stride partition dims), so the whole
voxel phase is a handful of VectorE tensor_tensor ops; no PE matmuls
outside the convolutions.  Each unique [128, 1024] f32 tile is stored
4x (t-dup x x-dup) with y/z duplication folded into the DMA access
patterns (4 KiB contiguous runs).
"""

import numpy as np

T2, X2, Y2, Z2, C = 8, 4, 32, 16, 16
NCORES = 8
CIN = 64

_CACHE = {}


def _build_program():
    from contextlib import ExitStack

    import concourse.bacc as bacc
    import concourse.bass as bass
    import concourse.mybir as mybir
    from concourse.tile import TileContext

    f32 = mybir.dt.float32
    AF = mybir.ActivationFunctionType
    MUL = mybir.AluOpType.mult
    AP = bass.AP

    nc = bacc.Bacc()
    ctx = ExitStack()

    # ---- external IO ----
    # One packed fp16 input: rows 0..63 = cin, row 64 = ones (bias channel).
    # Column segments: xyT[0:206] xz[206:316] yz[316:930] tx[930:992]
    # tyT[992:1334] tz[1334:1516] w[1516:2380]; convs read 3x3 windows,
    # w holds (plane, dy, dx, cout) with the bias in row 64 of the center tap.
    f16 = mybir.dt.float16
    KP = CIN + 1
    img_all = nc.dram_tensor("img_all", [KP, 2380], f16, kind="ExternalInput")
    out_d = nc.dram_tensor("out", [2 * T2, 2 * X2, 2 * Y2, 2 * Z2, C], f32,
                           kind="ExternalOutput")
    SEG = {"xyT": 0, "xz": 206, "yz": 316, "tx": 930, "tyT": 992,
           "tz": 1334, "w": 1516}

    # ---- DRAM scratch: raw conv-output dumps (flat [m*16]) ----
    yz_rows = [(0, 7), (7, 7), (14, 7), (21, 7), (28, 4)]
    ty_rows = [(0, 12), (12, 12), (24, 8)]
    edump = {}
    xy_rows = [(0, 21), (21, 11)]
    for k, m in ([("xz", 72), ("tx", 48), ("tz0", 72), ("tz1", 72)]
                 + [(f"xy{b}", nr * 6) for b, (r0, nr) in enumerate(xy_rows)]
                 + [(f"yz{b}", nr * 18) for b, (r0, nr) in enumerate(yz_rows)]
                 + [(f"ty{b}", nr * 10) for b, (r0, nr) in enumerate(ty_rows)]):
        edump[k] = nc.dram_tensor(f"e_{k}", [m * 16], f32)
    quD = nc.dram_tensor("quD", [32 * 256], f32)  # (x2, t2, z2, c) flat

    with TileContext(nc) as tc:
        sb = lambda name, shape: ctx.enter_context(
            nc.sbuf_tensor(name, shape, f32))
        # inputs (single packed fp16 tile)
        i_all = ctx.enter_context(nc.sbuf_tensor("i_all", [KP, 2380], f16))
        # voxel operands (partitions p = y2*4 + x2 unless noted)
        uxy_sb = sb("uxy_sb", [128, 16])      # p: c
        uxz_rep = sb("uxz_rep", [128, 256])   # p: (z2, c)  [rep over y2]
        uyz_rep = sb("uyz_rep", [128, 256])   # p: (z2, c)  [rep over x2]
        uty_rep = sb("uty_rep", [128, 128])   # p: (t2, c)  [rep over x2]
        qu_rep = sb("qu_rep", [128, 2048])    # p: (t2, z2, c)  [rep over y2]
        utx_sb = sb("utx_sb", [32, 16])       # p=(t2,x2): c
        utz_sb = sb("utz_sb", [32, 256])      # p=(t2,x2): (z2, c)
        qu_sb = sb("qu_sb", [32, 256])        # p=(t2,x2): (z2, c)
        m1a = sb("m1a", [128, 256])
        m1u = sb("m1u", [128, 256])
        tmp_all = sb("tmp_all", [128, 2048])  # p: (t2, z2, c) = m1u * ty

        # ---------- phase A: input load ----------
        nc.sync.dma_start(i_all[:], img_all[:])

        # ---------- phase B: convolutions ----------
        def wslice(i, dy, dx):
            off = SEG["w"] + ((i * 3 + dy) * 3 + dx) * 16
            return AP(i_all, off, [[2380, KP], [1, 16]])

        conv_pool_cm = tc.tile_pool(name="convpsum", bufs=2, space="PSUM")
        conv_pool = conv_pool_cm.__enter__()

        conv_outs = {}

        def conv_spatial(i, seg, wp, rows, row0, tag):
            # Full-width contiguous windows; junk at cols wp-2, wp-1.
            m = rows * wp
            psum = conv_pool.tile([m, 16], f32, name=f"cp_{tag}", tag="cp")
            for dy in range(3):
                for dx in range(3):
                    lhsT = AP(i_all, SEG[seg] + (row0 + dy) * wp + dx,
                              [[2380, KP], [1, m]])
                    nc.tensor.matmul(psum, lhsT, wslice(i, dy, dx),
                                     start=(dy == 0 and dx == 0),
                                     stop=(dy == 2 and dx == 2))
            out_sb = sb(f"c_{tag}", [m, 16])
            nc.scalar.activation(out_sb[:], psum, AF.Relu)
            conv_outs[tag] = out_sb

        dump_insts = {}

        def dump(eng, k):
            dump_insts[k] = eng.dma_start(edump[k][:], conv_outs[k][:])

        def reload(eng, deps, dst_ap, src_ap):
            inst = eng.dma_start(dst_ap, src_ap)
            for d in deps:
                bass._add_dep_helper(inst.ins, dump_insts[d].ins,
                                     reason=f"raw {d}")
            return inst

        # --- Q path first: tx, tz, then ty ---
        conv_spatial(3, "tx", 6, 8, 0, "tx")               # m=48
        dump(nc.sync, "tx")
        for k in range(2):
            conv_spatial(5, "tz", 18, 4, 4 * k, f"tz{k}")  # m=72
            dump(nc.sync, f"tz{k}")
        for b, (r0, nr) in enumerate(ty_rows):
            conv_spatial(4, "tyT", 10, nr, r0, f"ty{b}")
            dump(nc.sync, f"ty{b}")

        # Q = utx * utz on p=(t2, x2)
        reload(nc.sync, ["tx"], utx_sb[:],
               AP(edump["tx"], 0, [[96, 8], [16, 4], [1, 16]]))
        for k in range(2):
            reload(nc.sync, [f"tz{k}"],
                   AP(utz_sb, k * 16 * 256, [[256, 16], [1, 256]]),
                   AP(edump[f"tz{k}"], 0, [[288, 4], [0, 4], [1, 256]]))
        nc.vector.tensor_tensor(
            qu_sb[:], utz_sb[:], AP(utx_sb, 0, [[16, 32], [0, 16], [1, 16]]),
            MUL)
        # store as (x2, t2, z2, c) so the replicated reload is contiguous
        qu_store = nc.sync.dma_start(
            AP(quD, 0, [[256, 8], [2048, 4], [1, 256]]), qu_sb[:])
        qu_load = nc.sync.dma_start(
            qu_rep[:], AP(quD, 0, [[0, 32], [2048, 4], [1, 2048]]))
        bass._add_dep_helper(qu_load.ins, qu_store.ins, reason="raw quD")

        # uty_rep[p=(y2,x2), (t2, c)] from transposed-ty dumps
        for b, (r0, nr) in enumerate(ty_rows):
            reload(nc.sync, [f"ty{b}"],
                   AP(uty_rep, r0 * 4 * 128, [[128, 4 * nr], [1, 128]]),
                   AP(edump[f"ty{b}"], 0, [[160, nr], [0, 4], [1, 128]]))

        # --- M1 path: xy (transposed plane -> y2-major rows), xz, yz ---
        for b, (r0, nr) in enumerate(xy_rows):
            conv_spatial(0, "xyT", 6, nr, r0, f"xy{b}")
            dump(nc.scalar, f"xy{b}")
            reload(nc.scalar, [f"xy{b}"],
                   AP(uxy_sb, r0 * 4 * 16, [[16, 4 * nr], [1, 16]]),
                   AP(edump[f"xy{b}"], 0, [[96, nr], [16, 4], [1, 16]]))

        conv_spatial(1, "xz", 18, 4, 0, "xz")             # m=72
        dump(nc.scalar, "xz")
        reload(nc.scalar, ["xz"], uxz_rep[:],
               AP(edump["xz"], 0, [[0, 32], [288, 4], [1, 256]]))
        for b, (r0, nr) in enumerate(yz_rows):
            conv_spatial(2, "yz", 18, nr, r0, f"yz{b}")
            dump(nc.scalar, f"yz{b}")
            reload(nc.scalar, [f"yz{b}"],
                   AP(uyz_rep, r0 * 4 * 256, [[256, 4 * nr], [1, 256]]),
                   AP(edump[f"yz{b}"], 0, [[288, nr], [0, 4], [1, 256]]))

        conv_pool_cm.__exit__(None, None, None)

        # ---------- phase C: M1 and ty products ----------
        nc.vector.tensor_tensor(m1a[:], uxz_rep[:], uyz_rep[:], MUL)
        nc.vector.tensor_tensor(
            m1u[:], m1a[:], AP(uxy_sb, 0, [[16, 128], [0, 16], [1, 16]]), MUL)
        # tmp_all[p, (t2, z2, c)] = m1u[p, (z2, c)] * uty_rep[p, (t2, c)]
        nc.vector.tensor_tensor(
            tmp_all[:],
            AP(m1u, 0, [[256, 128], [0, 8], [16, 16], [1, 16]]),
            AP(uty_rep, 0, [[128, 128], [16, 8], [0, 16], [1, 16]]), MUL)

        # ---------- phase D: per-t2 voxel tiles + duplicated stores ----------
        from contextlib import ExitStack as _ES
        pool_ctx = _ES()
        out_pool = pool_ctx.enter_context(tc.tile_pool(name="outsb", bufs=3))

        for t2 in range(T2):
            o = out_pool.tile([128, 1024], f32, name="o", tag="o")
            op = o.ap[0][0]
            # o[p, (z2, zd, c)] = tmp_all[p, t2, z2, c] * qu_rep[p, t2, z2, c]
            nc.vector.tensor_tensor(
                AP(o.tensor, o.offset, [[op, 128], [32, 16], [16, 2], [1, 16]]),
                AP(tmp_all, t2 * 256, [[2048, 128], [16, 16], [0, 2], [1, 16]]),
                AP(qu_rep, t2 * 256, [[2048, 128], [16, 16], [0, 2], [1, 16]]),
                MUL)
            # duplicate the (z, c) half-row for the y-duplication run
            nc.vector.tensor_copy(
                AP(o.tensor, o.offset + 512, [[op, 128], [1, 512]]),
                AP(o.tensor, o.offset, [[op, 128], [1, 512]]))
            for td in range(2):
                for xd in range(2):
                    eng = nc.sync if (td * 2 + xd) % 2 == 0 else nc.scalar
                    dst = AP(out_d,
                             (2 * t2 + td) * 262144 + xd * 32768,
                             [[1024, 32], [65536, 4], [1, 1024]])
                    eng.dma_start(dst, o[:])

        pool_ctx.close()

    nc.compile()
    return nc, ctx


def _prep_inputs(plane_xy, plane_xz, plane_yz, plane_tx, plane_ty, plane_tz,
                 W, b):
    """Host-side slicing/padding/transposition into one packed fp16 input."""
    f32 = np.float32
    xy = np.asarray(plane_xy, f32)[0]  # [64, X'32, Y'32]
    xz = np.asarray(plane_xz, f32)[0]  # [64, X'32, Z'16]
    yz = np.asarray(plane_yz, f32)[0]  # [64, Y'32, Z'16]
    tx = np.asarray(plane_tx, f32)[0]  # [64, T'8,  X'32]
    ty = np.asarray(plane_ty, f32)[0]  # [64, T'8,  Y'32]
    tz = np.asarray(plane_tz, f32)[0]  # [64, T'8,  Z'16]
    W = np.asarray(W, f32)             # [6, 16, 64, 3, 3]
    b = np.asarray(b, f32)             # [6, 16]

    # xy and ty are convolved on transposed planes -> swap their 3x3 taps
    W2 = W.copy()
    W2[0] = W[0].transpose(0, 1, 3, 2)
    W2[4] = W[4].transpose(0, 1, 3, 2)
    # weight block [65, 864]: rows 0..63 = (ci, i, dy, dx, co); row 64 holds
    # the bias in the center tap (the ones-channel contributes it once).
    wseg = np.zeros((65, 864), f32)
    wseg[:64] = W2.transpose(2, 0, 3, 4, 1).reshape(CIN, 864)
    for i in range(6):
        wseg[64, ((i * 3 + 1) * 3 + 1) * 16:((i * 3 + 1) * 3 + 1) * 16 + 16] = b[i]

    def flat2(p):
        q = p.reshape(p.shape[0], -1)
        return np.ascontiguousarray(np.pad(q, ((0, 0), (0, 2))))

    def with_ones(img):
        return np.concatenate([img, np.ones((1, img.shape[1]), f32)], axis=0)

    img_yz = flat2(np.pad(yz, ((0, 0), (1, 1), (1, 1))))
    img_tyT = flat2(np.pad(ty.transpose(0, 2, 1), ((0, 0), (1, 1), (1, 1))))
    img_tz = flat2(np.pad(tz, ((0, 0), (1, 1), (1, 1))))

    def row_halo(p, x0h):
        out = np.zeros((p.shape[0], 6, p.shape[2]), f32)
        lo = x0h - 1
        s0, s1 = max(lo, 0), min(lo + 6, p.shape[1])
        out[:, s0 - lo:s0 - lo + (s1 - s0), :] = p[:, s0:s1, :]
        return out

    def col_halo(p, x0h):
        out = np.zeros((p.shape[0], p.shape[1], 6), f32)
        lo = x0h - 1
        s0, s1 = max(lo, 0), min(lo + 6, p.shape[2])
        out[:, :, s0 - lo:s0 - lo + (s1 - s0)] = p[:, :, s0:s1]
        return out

    in_maps = []
    for k in range(NCORES):
        x0h = 4 * k
        segs = [
            flat2(np.pad(col_halo(xy.transpose(0, 2, 1), x0h),
                         ((0, 0), (1, 1), (0, 0)))),            # xyT 206
            flat2(np.pad(row_halo(xz, x0h), ((0, 0), (0, 0), (1, 1)))),  # 110
            img_yz,                                             # 614
            flat2(np.pad(col_halo(tx, x0h), ((0, 0), (1, 1), (0, 0)))),  # 62
            img_tyT,                                            # 342
            img_tz,                                             # 182
        ]
        img = np.concatenate([with_ones(s) for s in segs] + [wseg], axis=1)
        in_maps.append({"img_all": img.astype(np.float16)})
    return in_maps


def kernel(plane_xy, plane_xz, plane_yz, plane_tx, plane_ty, plane_tz, W, b):
    from concourse.bass_utils import run_bass_kernel_spmd

    if "nc" not in _CACHE:
        _CACHE["nc"], _CACHE["ctx"] = _build_program()
    nc = _CACHE["nc"]

    in_maps = _prep_inputs(plane_xy, plane_xz, plane_yz, plane_tx, plane_ty,
                           plane_tz, W, b)
    res = run_bass_kernel_spmd(nc, in_maps, list(range(NCORES)))
    slices = [res.results[k]["out"] for k in range(NCORES)]
    full = np.concatenate(slices, axis=1)  # [T, 64, Y, Z, C]
    return full[None].astype(np.float32)
